# revision 2
# baseline (speedup 1.0000x reference)
"""Causal self-attention (RoPE) Trainium2 Bass kernel, 8-core SPMD.

Sharding: core c -> batch c//2, head-group c%2 (8 of 16 heads).
Per core: q/k/v projections column-sharded over heads, block-causal attention
for its 8 heads, out-projection row-sharded; the host sums the two partial
outputs per batch and adds bo.

Schedule: the v projection runs first (overlapping the x DMA stream), then
q/k projection for head-pair m+1 is interleaved INTO head-pair m's attention
one (qtr, q|k) group per attention chunk, so the PE, scalar (exp) and DVE
(rope) queues all stay loaded across what would otherwise be phase
boundaries. Attention per (head, 512-q chunk) is software-pipelined over
kv-tile duos: two score matmuls per [128, 1024] PSUM tile (s^T [kv, q]
layout), exp on the scalar engine, causal mask as a 0/1 indicator multiply
on pt after exp (DVE bf16, diag duos only), pv trailing two duos behind.
Softmax denominators come from a ones-column in v_ext (yt row 64);
normalization (yt copy, per-head batched reciprocal, gpsimd partition
broadcast, yn multiply) drips through an action queue one step per chunk
boundary so the DVE queue never head-of-line blocks.

PE pstate note (measured): back-to-back matmuls pipeline at full rate with
~100ns/instr overhead; any PE idle gap restarts the clock ramp at 0.65 GHz.
q/k row-blocks are duplicated across both partition halves so the qk
stationary covers all 128 rows (s comes out doubled; the 0.5 is folded into
exp's scale immediate).
"""
import sys

sys.path.insert(0, "/opt/trn_rl_repo")

import math
from contextlib import ExitStack

import ml_dtypes
import numpy as np

import concourse.bass as bass
import concourse.tile as tile
from concourse import bacc, mybir
from concourse.bass_utils import run_bass_kernel_spmd

F32 = mybir.dt.float32
F32R = mybir.dt.float32r
BF16 = mybir.dt.bfloat16
AF = mybir.ActivationFunctionType

N_CORES = 8
B, T, D = 4, 2048, 1024
H, HD = 16, 64          # total heads, head dim
HC = 8                  # heads per core
DC = HC * HD            # 512 sharded projection dims per core
BASE = 10000
NT = T // 128           # 16 t-tiles
NM = DC // 128          # 4 m-tiles of q/k (2 heads each)
NK = D // 128           # 8 contraction tiles of D
VW = HC * (HD + 1)      # 520: v_ext width (64 dims + ones col per head)


def _build_program():
    nc = bacc.Bacc("TRN2", target_bir_lowering=False, debug=False,
                   num_devices=N_CORES)

    def din(name, shape, dt=F32R):
        return nc.dram_tensor(name, shape, dt, kind="ExternalInput").ap()

    xT = din("xT", [D, T], BF16)              # x[b].T
    wqT = din("wqT", [D, DC], BF16)           # (Wq/8)[rows].T
    wkT = din("wkT", [D, DC], BF16)
    wvT = din("wvT", [D, DC], BF16)
    woT = din("woT", [DC, D], BF16)           # Wo[:, rows].T
    bqk_cols = din("bqk_cols", [128, 8], F32)  # q bias m-cols 0-3, k bias 4-7
    bv_row = din("bv_row", [1, DC], F32)
    ones_col = din("ones_col", [128, 8], F32)  # ones block for v_ext columns
    cosS = din("cosS", [128, T], BF16)        # 2-head-stacked cos table
    sinS = din("sinS", [128, T], BF16)        # sign-folded sin table
    vmask = din("vmask", [2, 128, 1024], BF16)  # causal KEEP indicator
    outT = nc.dram_tensor("outT", [D, T], F32, kind="ExternalOutput").ap()

    # round-robin DMA issue over engine queues to parallelize HBM fetch
    dq = []

    def dma(dst, src):
        q = dq[0]
        dq.append(dq.pop(0))
        q.dma_start(dst, src)

    with tile.TileContext(nc) as tc, ExitStack() as top:
        dq.extend([nc.sync, nc.scalar, nc.gpsimd])
        p_const = top.enter_context(tc.tile_pool(name="const", bufs=1))
        bqk_t = p_const.tile([128, 8], F32, name="bqk_t")
        dma(bqk_t[:], bqk_cols[:])
        bv_t = p_const.tile([1, DC], F32, name="bv_t")
        dma(bv_t[:], bv_row[:])
        bvb = p_const.tile([128, DC], F32, name="bvb")
        nc.gpsimd.partition_broadcast(bvb[:], bv_t[:])
        ones_col_t = p_const.tile([128, 8], F32, name="ones_col_t")
        dma(ones_col_t[:], ones_col[:])

        p_yn = top.enter_context(tc.tile_pool(name="yn", bufs=1))
        yn = [p_yn.tile([128, T], BF16, name=f"yn{m}") for m in range(NM)]
        p_v = top.enter_context(tc.tile_pool(name="pv_ext", bufs=1))
        v_ext = [p_v.tile([128, VW], BF16, name=f"vext{tt}")
                 for tt in range(NT)]
        p_wo = top.enter_context(tc.tile_pool(name="po_w", bufs=1))

        with tc.tile_pool(name="pw", bufs=1) as p_w, \
             tc.tile_pool(name="px", bufs=1) as p_x, \
             tc.tile_pool(name="ptmp", bufs=3) as p_tmp, \
             tc.tile_pool(name="pqk", bufs=2) as p_qk, \
             tc.tile_pool(name="pa_dup", bufs=2) as p_dup, \
             tc.tile_pool(name="pa_pt", bufs=3) as p_pt, \
             tc.tile_pool(name="pa_y", bufs=10) as p_y, \
             tc.tile_pool(name="pa_z", bufs=2) as p_z, \
             tc.tile_pool(name="po_st", bufs=3) as p_st, \
             tc.tile_pool(name="ps_acc", bufs=2, space="PSUM") as ps_acc, \
             tc.tile_pool(name="pa_s", bufs=2, space="PSUM") as ps_s, \
             tc.tile_pool(name="pa_yt", bufs=2, space="PSUM") as ps_yt:
            # ---- input DMAs, first-needed-first ----
            wq, wk, wv = [], [], []
            xs = [[None] * 4 for _ in range(NK)]
            for k in range(NK):
                xt = p_x.tile([128, 512], BF16, name=f"xs{k}q0")
                dma(xt[:], xT[bass.ts(k, 128), 0:512])
                xs[k][0] = xt
                wt = p_w.tile([128, DC], BF16, name=f"wv{k}")
                dma(wt[:], wvT[bass.ts(k, 128), :])
                wv.append(wt)
            for qtr in range(1, 4):
                for k in range(NK):
                    xt = p_x.tile([128, 512], BF16, name=f"xs{k}q{qtr}")
                    dma(xt[:], xT[bass.ts(k, 128), bass.ts(qtr, 512)])
                    xs[k][qtr] = xt
            for k in range(NK):
                wt = p_w.tile([128, DC], BF16, name=f"wq{k}")
                dma(wt[:], wqT[bass.ts(k, 128), :])
                wq.append(wt)
            for k in range(NK):
                wt = p_w.tile([128, DC], BF16, name=f"wk{k}")
                dma(wt[:], wkT[bass.ts(k, 128), :])
                wk.append(wt)
            cos_t = p_const.tile([128, T], BF16, name="cos_t")
            dma(cos_t[:], cosS[:])
            sin_t = p_const.tile([128, T], BF16, name="sin_t")
            dma(sin_t[:], sinS[:])
            vm = []
            for r in range(2):
                t = p_const.tile([128, 1024], BF16, name=f"vm{r}")
                dma(t[:], vmask[r])
                vm.append(t)
            wo = []
            for k in range(NM):
                wt = p_wo.tile([128, D], BF16, name=f"wo{k}")
                nc.sync.dma_start(wt[:], woT[bass.ts(k, 128), :])
                wo.append(wt)

            # ---- v projection (overlaps the x DMA stream) ----
            for tt in range(NT):
                qtr, tl = tt // 4, tt % 4
                acc = ps_acc.tile([128, DC], F32, name="acc_v", tag="acc",
                                  bufs=2)
                for k in range(NK):
                    nc.tensor.matmul(acc[:], xs[k][qtr][:, bass.ts(tl, 128)],
                                     wv[k][:], start=(k == 0),
                                     stop=(k == NK - 1))
                v3 = v_ext[tt][:].rearrange("p (h w) -> p h w", w=HD + 1)
                nc.gpsimd.tensor_copy(
                    v3[:, :, HD:HD + 1],
                    ones_col_t[:].rearrange("p (h w) -> p h w", w=1))
                nc.vector.tensor_add(
                    v3[:, :, 0:HD],
                    acc[:].rearrange("p (h w) -> p h w", w=HD),
                    bvb[:].rearrange("p (h w) -> p h w", w=HD))

            # ---- q/k projection group: one (qtr, q|k) eighth of m ----
            def proj_group(m, dest_pair, g):
                qtr, wi = g // 2, g % 2
                hs = bass.ts(qtr, 512)
                wlist = (wq, wk)[wi]
                dest = dest_pair[wi]
                acc = ps_acc.tile([128, 512], F32, name="acc_p", tag="acc",
                                  bufs=2)
                for k in range(NK):
                    nc.tensor.matmul(acc[:], wlist[k][:, bass.ts(m, 128)],
                                     xs[k][qtr][:], start=(k == 0),
                                     stop=(k == NK - 1))
                qb = p_tmp.tile([128, 512], BF16, name="rope_qb",
                                tag="rope_qb", bufs=3)
                nc.vector.tensor_scalar_add(
                    qb[:], acc[:], bqk_t[:, 4 * wi + m:4 * wi + m + 1])
                shuf = p_tmp.tile([128, 512], BF16, name="rope_shuf",
                                  tag="rope_shuf", bufs=3)
                for (dst, src) in ((0, 32), (32, 0), (64, 96), (96, 64)):
                    nc.gpsimd.dma_start(shuf[dst:dst + 32, :],
                                        qb[src:src + 32, :])
                t1 = p_tmp.tile([128, 512], BF16, name="rope_t1",
                                tag="rope_t1", bufs=3)
                nc.vector.tensor_mul(t1[:], qb[:], cos_t[:, hs])
                nc.vector.tensor_mul(shuf[:], shuf[:], sin_t[:, hs])
                nc.vector.tensor_add(dest[:, hs], t1[:], shuf[:])

            def new_qk(m):
                qTm = p_qk.tile([128, T], BF16, name="qTm", tag="qT", bufs=2)
                kTm = p_qk.tile([128, T], BF16, name="kTm", tag="kT", bufs=2)
                return (qTm, kTm)

            def dup_head(qk_pair, hh):
                qTm, kTm = qk_pair
                prow = slice(64 * hh, 64 * hh + 64)
                kTd = p_dup.tile([128, T], BF16, name="kTd", tag="kTd",
                                 bufs=2)
                qTd = p_dup.tile([128, T], BF16, name="qTd", tag="qTd",
                                 bufs=2)
                for half in range(2):
                    nc.sync.dma_start(kTd[bass.ts(half, 64), :], kTm[prow, :])
                    nc.sync.dma_start(qTd[bass.ts(half, 64), :], qTm[prow, :])
                return kTd, qTd

            # deferred normalization machinery
            zall = [None]
            ysbs = []
            pend_copy = [None]
            norm_actions = []

            def make_norm(m_p, hh_p, J_p, ysb, zrec):
                def act():
                    zr1 = p_z.tile([1, 512], F32, name="zr1", tag="zr1",
                                   bufs=2)
                    nc.sync.dma_start(zr1[:], zrec[J_p:J_p + 1, :])
                    zb = p_z.tile([64, 512], F32, name="zb", tag="zb",
                                  bufs=2)
                    nc.gpsimd.partition_broadcast(zb[:], zr1[:])
                    nc.vector.tensor_mul(
                        yn[m_p][64 * hh_p:64 * hh_p + 64, bass.ts(J_p, 512)],
                        ysb[0:64, :], zb[:])
                return act

            def make_recip(zall_t, entries):
                def act():
                    zrec = p_z.tile([4, 512], F32, name="zrec", tag="zrec",
                                    bufs=2)
                    nc.vector.reciprocal(zrec[:], zall_t[:])
                    for (m2, hh2, J2, ysb2) in entries:
                        norm_actions.append(
                            make_norm(m2, hh2, J2, ysb2, zrec))
                return act

            def chunk_boundary():
                if pend_copy[0] is not None:
                    yt_p, m_p, hh_p, J_p = pend_copy[0]
                    pend_copy[0] = None
                    ysb = p_y.tile([65, 512], F32, name="ysb", tag="ysb",
                                   bufs=10)
                    nc.vector.tensor_copy(ysb[:], yt_p[:])
                    if zall[0] is None:
                        zall[0] = p_z.tile([4, 512], F32, name="zall",
                                           tag="zall", bufs=2)
                    nc.sync.dma_start(zall[0][J_p:J_p + 1, :],
                                      ysb[64:65, :])
                    ysbs.append((m_p, hh_p, J_p, ysb))
                    if len(ysbs) == 4:
                        norm_actions.append(make_recip(zall[0], list(ysbs)))
                        ysbs.clear()
                        zall[0] = None
                n = 2 if len(norm_actions) >= 3 else 1
                for _ in range(n):
                    if norm_actions:
                        norm_actions.pop(0)()

            def out_group(n):
                for M in range(NK):
                    acc = ps_acc.tile([128, 512], F32, name="acc_o",
                                      tag="acc", bufs=2)
                    for k in range(NM):
                        nc.tensor.matmul(acc[:], wo[k][:, bass.ts(M, 128)],
                                         yn[k][:, bass.ts(n, 512)],
                                         start=(k == 0), stop=(k == NM - 1))
                    st = p_st.tile([128, 512], F32, name="out_st", tag="st",
                                   bufs=3)
                    if M % 2 == 0:
                        nc.vector.tensor_copy(st[:], acc[:])
                    else:
                        nc.scalar.activation(st[:], acc[:], AF.Identity)
                    nc.gpsimd.dma_start(
                        outT[bass.ts(M, 128), bass.ts(n, 512)], st[:])

            # ---- interleaved projection + attention over head-pairs ----
            cur_qk = new_qk(0)
            for g in range(8):
                proj_group(0, cur_qk, g)
            dups = dup_head(cur_qk, 0)
            next_qk = None
            # groups of m+1 all emitted during m's even head, so the next
            # head-pair's dup can issue at the odd head's top (full-head lead)
            PACE = (2, 2, 2, 2, 0, 0, 0, 0)

            for h in range(HC):
                m, hh = h // 2, h % 2
                kTd, qTd = dups
                if hh == 0 and m < NM - 1:
                    next_qk = new_qk(m + 1)
                # dup for the next head at head top: even heads dup the same
                # pair's hh=1 (projected long ago); odd heads dup the next
                # pair, whose proj groups all ran during the even head.
                if h < HC - 1:
                    if hh == 0:
                        dups_next = dup_head(cur_qk, 1)
                    else:
                        cur_qk = next_qk
                        dups_next = dup_head(cur_qk, 0)
                gi = sum(PACE[:4 * hh])
                for J in range(4):
                    qs = bass.ts(J, 512)
                    yt = ps_yt.tile([65, 512], F32, name="yt", tag="yt",
                                    bufs=2)
                    pend = []

                    def emit_pv(ent):
                        d, pt = ent
                        for half in range(2):
                            kvt = 2 * d + half
                            v3 = v_ext[kvt][:].rearrange(
                                "p (h w) -> p h w", w=HD + 1)
                            nc.tensor.matmul(
                                yt[:], v3[:, h, :],
                                pt[:, bass.ts(half, 512)],
                                start=(kvt == 0), stop=(kvt == 4 * J + 3))

                    for d in range(2 * J + 2):
                        sb = ps_s.tile([128, 1024], F32, name="sb",
                                       tag="sb", bufs=2)
                        for half in range(2):
                            nc.tensor.matmul(
                                sb[:, bass.ts(half, 512)],
                                kTd[:, bass.ts(2 * d + half, 128)],
                                qTd[:, qs], start=True, stop=True)
                        pt = p_pt.tile([128, 1024], BF16, name="pt",
                                       tag="pt", bufs=3)
                        nc.scalar.activation(pt[:], sb[:], AF.Exp, scale=0.5)
                        if d >= 2 * J:
                            # diag duo: multiply in the 0/1 causal keep mask
                            nc.vector.tensor_mul(pt[:], pt[:],
                                                 vm[d - 2 * J][:])
                        pend.append((d, pt))
                        if len(pend) > 2:
                            emit_pv(pend.pop(0))
                    for ent in pend:
                        emit_pv(ent)
                    chunk_boundary()
                    if h >= HC - 2:
                        # last head: immediate per-chunk normalize so no
                        # batched drain serializes against the out projection
                        zrec1 = p_z.tile([1, 512], F32, name="zrec1",
                                         tag="zrec1", bufs=2)
                        nc.vector.reciprocal(zrec1[:], yt[64:65, :])
                        zb = p_z.tile([64, 512], F32, name="zb", tag="zb",
                                      bufs=2)
                        nc.gpsimd.partition_broadcast(zb[:], zrec1[:])
                        nc.vector.tensor_mul(
                            yn[m][64 * hh:64 * hh + 64, qs],
                            yt[0:64, :], zb[:])
                    else:
                        pend_copy[0] = (yt, m, hh, J)
                    if m < NM - 1:
                        for _ in range(PACE[4 * hh + J]):
                            proj_group(m + 1, next_qk, gi)
                            gi += 1
                    if h == HC - 1 and J >= 1:
                        # out-projection chunk J-1 is fully normalized now;
                        # overlap it under the last head's attention
                        out_group(J - 1)
                if h < HC - 1:
                    dups = dups_next
            # drain remaining normalization, then the last out chunk
            chunk_boundary()
            while norm_actions:
                norm_actions.pop(0)()
            out_group(3)

    nc.compile()
    return nc


_NC_CACHE = None


def _get_program():
    global _NC_CACHE
    if _NC_CACHE is None:
        _NC_CACHE = _build_program()
    return _NC_CACHE


def _host_inputs(x, Wq, bq, Wk, bk, Wv, bv, Wo, bo):
    scale = 1.0 / math.sqrt(HD)
    Wq_s = (np.asarray(Wq, dtype=np.float32) * scale).astype(np.float32)
    bq_s = (np.asarray(bq, dtype=np.float32) * scale).astype(np.float32)
    x = np.asarray(x, dtype=np.float32)
    Wk = np.asarray(Wk, dtype=np.float32)
    Wv = np.asarray(Wv, dtype=np.float32)
    Wo = np.asarray(Wo, dtype=np.float32)
    bk = np.asarray(bk, dtype=np.float32)
    bv = np.asarray(bv, dtype=np.float32)

    # rope tables, 2-head-stacked [128, T]
    j = np.arange(HD // 2, dtype=np.float64)
    theta = BASE ** (-2.0 * j / HD)                      # [32]
    pos = np.arange(1, T + 1, dtype=np.float64)          # [T]
    ang = pos[None, :] * theta[:, None]                  # [32, T]
    cos32 = np.cos(ang)
    sin32 = np.sin(ang)
    cos64 = np.concatenate([cos32, cos32], axis=0)       # [64, T]
    sin64 = np.concatenate([-sin32, sin32], axis=0)      # sign-folded
    cosS = np.concatenate([cos64, cos64], axis=0).astype(np.float32)
    sinS = np.concatenate([sin64, sin64], axis=0).astype(np.float32)

    p = np.arange(128)
    f = np.arange(512)
    vmask = np.zeros((2, 128, 1024), dtype=np.float32)
    for r in range(4):
        vmask[r // 2, :, 512 * (r % 2):512 * (r % 2) + 512] = (
            (128 * r + p[:, None]) <= f[None, :]).astype(np.float32)

    in_maps = []
    for c in range(N_CORES):
        b, g = c // 2, c % 2
        rows = slice(DC * g, DC * (g + 1))
        bqk = np.zeros((128, 8), dtype=np.float32)
        for m in range(NM):
            bqk[:, m] = bq_s[rows][128 * m:128 * (m + 1)]
            bqk[:, 4 + m] = bk[rows][128 * m:128 * (m + 1)]
        bf = ml_dtypes.bfloat16
        in_maps.append({
            "xT": np.ascontiguousarray(x[b].T).astype(bf),
            "wqT": np.ascontiguousarray(Wq_s[rows].T).astype(bf),
            "wkT": np.ascontiguousarray(Wk[rows].T).astype(bf),
            "wvT": np.ascontiguousarray(Wv[rows].T).astype(bf),
            "woT": np.ascontiguousarray(Wo[:, rows].T).astype(bf),
            "bqk_cols": bqk,
            "bv_row": bv[rows].reshape(1, DC),
            "ones_col": np.ones((128, 8), dtype=np.float32),
            "cosS": cosS.astype(bf),
            "sinS": sinS.astype(bf),
            "vmask": vmask.astype(bf),
        })
    return in_maps


def kernel(x, Wq, bq, Wk, bk, Wv, bv, Wo, bo, _trace=False, _tmpdir=None):
    nc = _get_program()
    in_maps = _host_inputs(x, Wq, bq, Wk, bk, Wv, bv, Wo, bo)
    res = run_bass_kernel_spmd(nc, in_maps, list(range(N_CORES)),
                               trace=_trace, tmpdir=_tmpdir)
    kernel.last_exec_time_ns = res.exec_time_ns
    bo = np.asarray(bo, dtype=np.float32)
    out = np.zeros((B, T, D), dtype=np.float32)
    for b in range(B):
        acc = res.results[2 * b]["outT"].astype(np.float32) + \
            res.results[2 * b + 1]["outT"].astype(np.float32)
        out[b] = acc.T + bo[None, :]
    return out



# revision 12
# speedup vs baseline: 1.0629x; 1.0629x over previous
"""Causal self-attention (RoPE) Trainium2 Bass kernel, 8-core SPMD.

Sharding: core c -> batch c//2, head-group c%2 (8 of 16 heads).
Per core: q/k/v projections column-sharded over heads, block-causal attention
for its 8 heads, out-projection row-sharded; the host sums the two partial
outputs per batch and adds bo.

All matmuls run in 64x128 row-tiled PE mode (tile_size (64,128), zero mode
switches). Scores for a head PAIR run as two concurrent row-tiles (T0 rows
0-63 = even head, T8 rows 64-127 = odd head; measured 109ns/MM vs 216 for
128-mode). Projections / pv / out-proj run "crosswise": two psum
accumulators in flight, T0 takes the low contraction half of one while T8
takes the high half of the other, then they swap (measured parity with
128-mode). Attention per (pair, 512-q chunk J) walks kv tiles of 128;
diagonal tiles (i-4J=r>=0) are narrowed to q columns [128r, 512) and the
remaining 128x128 triangle is masked by one bf16 multiply after exp.
Softmax denominators come from a ones-column in v_ext (row 64 of yt);
reciprocals are batched per pair into one [8,512] DVE reciprocal (cost is
free-dim proportional: 3.34us regardless of partition count); pair 3
normalizes per-chunk so the out-projection chunks can overlap its
attention. QK projection for pair m+1 and the v projection tail drip into
the ACT-paced attention stream one 4-MM granule at a time via a fill
queue; out-proj granules drip into pair 3.
"""
import sys

sys.path.insert(0, "/opt/trn_rl_repo")

import math
from contextlib import ExitStack

import ml_dtypes
import numpy as np

import concourse.bass as bass
import concourse.tile as tile
from concourse import bacc, mybir
from concourse.bass_utils import run_bass_kernel_spmd

F32 = mybir.dt.float32
BF16 = mybir.dt.bfloat16
AF = mybir.ActivationFunctionType

N_CORES = 8
B, T, D = 4, 2048, 1024
H, HD = 16, 64          # total heads, head dim
HC = 8                  # heads per core
DC = HC * HD            # 512 sharded projection dims per core
BASE = 10000
NT = T // 128           # 16 t-tiles
NM = DC // 128          # 4 m-tiles of q/k (2 heads each)
NK = D // 128           # 8 contraction tiles of D
NQ = T // 512           # 4 q-chunks
VW = HC * (HD + 1)      # 520: v_ext width (64 dims + ones col per head)


def _build_program():
    nc = bacc.Bacc("TRN2", target_bir_lowering=False, debug=False,
                   num_devices=N_CORES)

    def din(name, shape, dt):
        return nc.dram_tensor(name, shape, dt, kind="ExternalInput").ap()

    xT = din("xT", [D, T], BF16)              # x[b].T
    wqT = din("wqT", [D, DC], BF16)           # (Wq/8)[rows].T
    wkT = din("wkT", [D, DC], BF16)
    wvT = din("wvT", [D, DC], BF16)
    woT = din("woT", [DC, D], BF16)           # Wo[:, rows].T
    bqk_cols = din("bqk_cols", [128, 8], F32)  # q bias m-cols 0-3, k bias 4-7
    bv_row = din("bv_row", [1, DC], F32)
    ones_col = din("ones_col", [128, 8], F32)  # ones block for v_ext columns
    cosS = din("cosS", [128, T], BF16)        # 2-head-stacked cos table
    sinS = din("sinS", [128, T], BF16)        # sign-folded sin table
    triS = din("triS", [128, 256], BF16)      # [tri|tri] causal KEEP triangle
    outT = nc.dram_tensor("outT", [D, T], F32, kind="ExternalOutput").ap()

    # round-robin DMA issue over engine queues to parallelize HBM fetch
    dq = []

    def dma(dst, src):
        q = dq[0]
        dq.append(dq.pop(0))
        q.dma_start(dst, src)

    mm = nc.tensor.matmul

    with tile.TileContext(nc) as tc, ExitStack() as top:
        dq.extend([nc.sync, nc.scalar, nc.gpsimd])
        p_const = top.enter_context(tc.tile_pool(name="const", bufs=1))
        bqk_t = p_const.tile([128, 8], F32, name="bqk_t")
        dma(bqk_t[:], bqk_cols[:])
        bv_t = p_const.tile([1, DC], F32, name="bv_t")
        dma(bv_t[:], bv_row[:])
        bvb = p_const.tile([128, DC], F32, name="bvb")
        nc.gpsimd.partition_broadcast(bvb[:], bv_t[:])
        ones_col_t = p_const.tile([128, 8], F32, name="ones_col_t")
        dma(ones_col_t[:], ones_col[:])

        p_yn = top.enter_context(tc.tile_pool(name="yn", bufs=1))
        yn = [p_yn.tile([128, T], BF16, name=f"yn{m}") for m in range(NM)]
        p_v = top.enter_context(tc.tile_pool(name="pv_ext", bufs=1))
        v_ext = [p_v.tile([128, VW], BF16, name=f"vext{tt}")
                 for tt in range(NT)]
        p_wo = top.enter_context(tc.tile_pool(name="po_w", bufs=1))

        with tc.tile_pool(name="pw", bufs=1) as p_w, \
             tc.tile_pool(name="px", bufs=1) as p_x, \
             tc.tile_pool(name="ptmp", bufs=3) as p_tmp, \
             tc.tile_pool(name="pqk", bufs=1) as p_qk, \
             tc.tile_pool(name="ppt", bufs=4) as p_pt, \
             tc.tile_pool(name="pysb", bufs=8) as p_ysb, \
             tc.tile_pool(name="pz", bufs=2) as p_z, \
             tc.tile_pool(name="pst", bufs=3) as p_st, \
             tc.tile_pool(name="ps_acc", bufs=2, space="PSUM") as ps_acc, \
             tc.tile_pool(name="ps_sb", bufs=2, space="PSUM") as ps_sb, \
             tc.tile_pool(name="ps_yt", bufs=2, space="PSUM") as ps_yt:
            # ---- input DMAs, first-needed-first ----
            wq, wk, wv = [], [], []
            xs = [[None] * NQ for _ in range(NK)]
            for k in range(NK):
                xt = p_x.tile([128, 512], BF16, name=f"xs{k}q0")
                dma(xt[:], xT[bass.ts(k, 128), 0:512])
                xs[k][0] = xt
                wt = p_w.tile([128, DC], BF16, name=f"wv{k}")
                dma(wt[:], wvT[bass.ts(k, 128), :])
                wv.append(wt)
            for k in range(NK):
                wt = p_w.tile([128, DC], BF16, name=f"wq{k}")
                dma(wt[:], wqT[bass.ts(k, 128), :])
                wq.append(wt)
            for k in range(NK):
                wt = p_w.tile([128, DC], BF16, name=f"wk{k}")
                dma(wt[:], wkT[bass.ts(k, 128), :])
                wk.append(wt)
            cos_t = p_const.tile([128, T], BF16, name="cos_t")
            dma(cos_t[:], cosS[:])
            sin_t = p_const.tile([128, T], BF16, name="sin_t")
            dma(sin_t[:], sinS[:])
            tri_t = p_const.tile([128, 256], BF16, name="tri_t")
            dma(tri_t[:], triS[:])
            for qtr in range(1, NQ):
                for k in range(NK):
                    xt = p_x.tile([128, 512], BF16, name=f"xs{k}q{qtr}")
                    dma(xt[:], xT[bass.ts(k, 128), bass.ts(qtr, 512)])
                    xs[k][qtr] = xt
            wo = []
            for k in range(NM):
                wt = p_wo.tile([128, D], BF16, name=f"wo{k}")
                nc.sync.dma_start(wt[:], woT[bass.ts(k, 128), :])
                wo.append(wt)

            qTm = [p_qk.tile([128, T], BF16, name=f"qTm{m}")
                   for m in range(NM)]
            kTm = [p_qk.tile([128, T], BF16, name=f"kTm{m}")
                   for m in range(NM)]

            # ---- crosswise generators (each yield = one 4-MM granule) ----
            def gen_v(tt):
                qtr, tl = divmod(tt, 4)
                acc = ps_acc.tile([128, DC], F32, name="acc_v", tag="acc",
                                  bufs=2)
                for k in range(NK):
                    mm(acc[:], xs[k][qtr][:, bass.ts(tl, 128)], wv[k][:],
                       start=(k == 0), stop=(k == NK - 1))
                    if k % 2 == 1:
                        yield
                v3 = v_ext[tt][:].rearrange("p (h w) -> p h w", w=HD + 1)
                nc.gpsimd.tensor_copy(
                    v3[:, :, HD:HD + 1],
                    ones_col_t[:].rearrange("p (h w) -> p h w", w=1))
                nc.vector.tensor_add(
                    v3[:, :, 0:HD],
                    acc[:].rearrange("p (h w) -> p h w", w=HD),
                    bvb[:].rearrange("p (h w) -> p h w", w=HD))

            def rope_emit(acc, dest, qtr, bcol):
                hs = bass.ts(qtr, 512)
                qb = p_tmp.tile([128, 512], BF16, name="rope_qb",
                                tag="rope_qb", bufs=3)
                nc.vector.tensor_scalar_add(
                    qb[:], acc[:], bqk_t[:, bcol:bcol + 1])
                shuf = p_tmp.tile([128, 512], BF16, name="rope_shuf",
                                  tag="rope_shuf", bufs=3)
                for (dst, src) in ((0, 32), (32, 0), (64, 96), (96, 64)):
                    nc.gpsimd.dma_start(shuf[dst:dst + 32, :],
                                        qb[src:src + 32, :])
                t1 = p_tmp.tile([128, 512], BF16, name="rope_t1",
                                tag="rope_t1", bufs=3)
                nc.vector.tensor_mul(t1[:], qb[:], cos_t[:, hs])
                nc.vector.tensor_mul(shuf[:], shuf[:], sin_t[:, hs])
                nc.vector.tensor_add(dest[:, hs], t1[:], shuf[:])

            def gen_qk_pair(m, qtr):
                mc = bass.ts(m, 128)
                accQ = ps_acc.tile([128, 512], F32, name="accQ", tag="acc",
                                   bufs=2)
                for k in range(NK):
                    mm(accQ[:], wq[k][:, mc], xs[k][qtr][:],
                       start=(k == 0), stop=(k == NK - 1))
                    if k % 2 == 1:
                        yield
                rope_emit(accQ, qTm[m], qtr, m)
                accK = ps_acc.tile([128, 512], F32, name="accK", tag="acc",
                                   bufs=2)
                for k in range(NK):
                    mm(accK[:], wk[k][:, mc], xs[k][qtr][:],
                       start=(k == 0), stop=(k == NK - 1))
                    if k % 2 == 1:
                        yield
                rope_emit(accK, kTm[m], qtr, 4 + m)

            def gen_out_pair(j2, J):
                qs = bass.ts(J, 512)
                for M in (2 * j2, 2 * j2 + 1):
                    acc = ps_acc.tile([128, 512], F32, name="acc_o",
                                      tag="acc", bufs=2)
                    for mk in range(NM):
                        mm(acc[:], wo[mk][:, bass.ts(M, 128)],
                           yn[mk][:, qs], start=(mk == 0),
                           stop=(mk == NM - 1))
                        if mk % 2 == 1:
                            yield
                    st = p_st.tile([128, 512], F32, name="out_st", tag="st",
                                   bufs=2)
                    if M % 2 == 0:
                        nc.vector.tensor_copy(st[:], acc[:])
                    else:
                        nc.scalar.activation(st[:], acc[:], AF.Identity)
                    nc.gpsimd.dma_start(outT[bass.ts(M, 128), qs], st[:])

            # ---- fill queues ----
            # fillq items: (need_key, generator); need_key = (m, J) means
            # "must be fully emitted before attention chunk A(m, J)".
            fillq = []
            outq = []    # out-proj generators (pair-3 / tail pops)
            dveq = []    # deferred normalization actions

            def pop_fill(n):
                for _ in range(n):
                    while fillq:
                        try:
                            next(fillq[0][1])
                            break
                        except StopIteration:
                            fillq.pop(0)
                    else:
                        while outq:
                            try:
                                next(outq[0])
                                break
                            except StopIteration:
                                outq.pop(0)
                        else:
                            return

            def drain_until(key):
                while fillq and fillq[0][0] <= key:
                    try:
                        next(fillq[0][1])
                    except StopIteration:
                        fillq.pop(0)

            def norm_one(m, hh, J, ysb, zrec_box):
                def act():
                    zr1 = p_z.tile([1, 512], F32, name="zr1", tag="zr1",
                                   bufs=2)
                    r = zrec_box[1][(m, hh, J)]
                    nc.sync.dma_start(zr1[:], zrec_box[0][r:r + 1, :])
                    zb = p_z.tile([64, 512], F32, name="zb", tag="zb",
                                  bufs=2)
                    nc.gpsimd.partition_broadcast(zb[:], zr1[:])
                    nc.vector.tensor_mul(
                        yn[m][64 * hh:64 * hh + 64, bass.ts(J, 512)],
                        ysb[0:64, :], zb[:])
                return act

            # ---- attention, pair-major ----
            # warmup: v tiles 0..3 + qk(0,0) emitted dense up front
            vgens = [gen_v(tt) for tt in range(NT)]
            for tt in range(4):
                for _ in vgens[tt]:
                    pass
            for _ in gen_qk_pair(0, 0):
                pass
            for qtr in range(1, NQ):
                # A(0, qtr) needs v_ext tiles <= 4*qtr+3 and qk(0, qtr)
                for tt in range(4 * qtr, 4 * qtr + 4):
                    fillq.append(((0, qtr), vgens[tt]))
                fillq.append(((0, qtr), gen_qk_pair(0, qtr)))
            for mn in range(1, NM):
                for qtr in range(NQ):
                    fillq.append(((mn, qtr), gen_qk_pair(mn, qtr)))

            for m in range(NM):
                last_pair = m == NM - 1
                if not last_pair:
                    zden_t = p_z.tile([8, 512], F32, name="zden",
                                      tag="zden", bufs=2)
                    pair_rows = []
                unit = 0
                for J in range(NQ):
                    drain_until((m, J))
                    ntiles = 4 * J + 4
                    yt0 = ps_yt.tile([65, 512], F32, name="yt0", tag="yt",
                                     bufs=2)
                    yt1 = ps_yt.tile([65, 512], F32, name="yt1", tag="yt",
                                     bufs=2)
                    pend = []

                    def emit_pv(ent, yt0=None, yt1=None, m=m, ntiles=None):
                        # 128-mode pv: full-kv contraction, one MM per head
                        i, pt_t, qlo = ent
                        v3 = v_ext[i][:].rearrange("p (h w) -> p h w",
                                                   w=HD + 1)
                        e0 = i == 0
                        el = i == ntiles - 1
                        mm(yt0[:, qlo:512], v3[:, 2 * m, :],
                           pt_t[:, qlo:512], start=e0, stop=el)
                        mm(yt1[:, qlo:512], v3[:, 2 * m + 1, :],
                           pt_t[:, 512 + qlo:1024], start=e0, stop=el)

                    # two score tiles per mode phase (64-mode pair bursts,
                    # then a 128-mode burst of pv + projection granules)
                    for i2 in range(0, ntiles, 2):
                        for i in (i2, i2 + 1):
                            r = i - 4 * J
                            qlo = 128 * r if r >= 0 else 0
                            lo, hi = 512 * J + qlo, 512 * J + 512
                            sb = ps_sb.tile([128, 1024], F32, name="sb",
                                            tag="sb", bufs=2)
                            mm(sb[:, qlo:512],
                               kTm[m][0:64, bass.ts(i, 128)],
                               qTm[m][0:64, lo:hi], start=True, stop=True)
                            mm(sb[:, 512 + qlo:1024],
                               kTm[m][64:128, bass.ts(i, 128)],
                               qTm[m][64:128, lo:hi], start=True, stop=True)
                            pt_t = p_pt.tile([128, 1024], BF16, name="pt",
                                             tag="pt", bufs=4)
                            sb3 = sb[:].rearrange("p (g w) -> p g w",
                                                  g=2)[:, :, qlo:512]
                            pt3 = pt_t[:].rearrange("p (g w) -> p g w",
                                                    g=2)[:, :, qlo:512]
                            nc.scalar.activation(pt3, sb3, AF.Exp,
                                                 scale=1.0)
                            if r >= 0:
                                ptm = pt_t[:].rearrange(
                                    "p (g w) -> p g w",
                                    g=2)[:, :, qlo:qlo + 128]
                                nc.vector.tensor_mul(
                                    ptm, ptm,
                                    tri_t[:].rearrange("p (g w) -> p g w",
                                                       g=2))
                            pend.append((i, pt_t, qlo))
                        for _ in range(2):
                            if len(pend) > 2:
                                emit_pv(pend.pop(0), yt0=yt0, yt1=yt1,
                                        ntiles=ntiles)
                        # drips: norm actions + projection granules
                        for _ in range(2):
                            if dveq:
                                dveq.pop(0)()
                                if last_pair and dveq:
                                    dveq.pop(0)()
                        pop_fill(4 if m == 0 else 2)
                        unit += 2
                    for ent in pend:
                        emit_pv(ent, yt0=yt0, yt1=yt1, ntiles=ntiles)
                    # pair/chunk epilogue: evacuate yt, gather denominators
                    ysbs = []
                    for hh, yt in ((0, yt0), (1, yt1)):
                        ysb = p_ysb.tile([65, 512], F32, name="ysb",
                                         tag="ysb", bufs=8)
                        nc.vector.tensor_copy(ysb[:], yt[:])
                        ysbs.append((hh, ysb))
                    if not last_pair:
                        for hh, ysb in ysbs:
                            nc.sync.dma_start(
                                zden_t[2 * J + hh:2 * J + hh + 1, :],
                                ysb[64:65, :])
                            pair_rows.append((hh, J, ysb))
                    else:
                        # immediate per-chunk normalize for the last pair
                        zden3 = p_z.tile([2, 512], F32, name="zden3",
                                         tag="zden3", bufs=2)
                        for hh, ysb in ysbs:
                            nc.sync.dma_start(zden3[hh:hh + 1, :],
                                              ysb[64:65, :])
                        zrec_box = [None, {}]

                        def recip3(zden3=zden3, zrec_box=zrec_box, J=J):
                            zrec = p_z.tile([2, 512], F32, name="zrec3",
                                            tag="zrec3", bufs=2)
                            nc.vector.reciprocal(zrec[:], zden3[:])
                            zrec_box[0] = zrec

                        dveq.append(recip3)
                        for hh, ysb in ysbs:
                            zrec_box[1][(m, hh, J)] = hh
                            dveq.append(norm_one(m, hh, J, ysb, zrec_box))
                        outq.append(gen_out_pair(0, J))
                        outq.append(gen_out_pair(1, J))
                        outq.append(gen_out_pair(2, J))
                        outq.append(gen_out_pair(3, J))
                if not last_pair:
                    # batched pair normalization, dripped into pair m+1
                    zrec_box = [None, {}]

                    def recip8(zden_t=zden_t, zrec_box=zrec_box):
                        zrec = p_z.tile([8, 512], F32, name="zrec",
                                        tag="zrec", bufs=2)
                        nc.vector.reciprocal(zrec[:], zden_t[:])
                        zrec_box[0] = zrec

                    dveq.append(recip8)
                    for hh, J, ysb in pair_rows:
                        zrec_box[1][(m, hh, J)] = 2 * J + hh
                        dveq.append(norm_one(m, hh, J, ysb, zrec_box))
            # ---- drain tail ----
            while dveq:
                dveq.pop(0)()
            pop_fill(10 ** 6)
            while outq:
                try:
                    next(outq[0])
                except StopIteration:
                    outq.pop(0)

    nc.compile()
    return nc


_NC_CACHE = None


def _get_program():
    global _NC_CACHE
    if _NC_CACHE is None:
        _NC_CACHE = _build_program()
    return _NC_CACHE


def _host_inputs(x, Wq, bq, Wk, bk, Wv, bv, Wo, bo):
    scale = 1.0 / math.sqrt(HD)
    Wq_s = (np.asarray(Wq, dtype=np.float32) * scale).astype(np.float32)
    bq_s = (np.asarray(bq, dtype=np.float32) * scale).astype(np.float32)
    x = np.asarray(x, dtype=np.float32)
    Wk = np.asarray(Wk, dtype=np.float32)
    Wv = np.asarray(Wv, dtype=np.float32)
    Wo = np.asarray(Wo, dtype=np.float32)
    bk = np.asarray(bk, dtype=np.float32)
    bv = np.asarray(bv, dtype=np.float32)

    # rope tables, 2-head-stacked [128, T]
    j = np.arange(HD // 2, dtype=np.float64)
    theta = BASE ** (-2.0 * j / HD)                      # [32]
    pos = np.arange(1, T + 1, dtype=np.float64)          # [T]
    ang = pos[None, :] * theta[:, None]                  # [32, T]
    cos32 = np.cos(ang)
    sin32 = np.sin(ang)
    cos64 = np.concatenate([cos32, cos32], axis=0)       # [64, T]
    sin64 = np.concatenate([-sin32, sin32], axis=0)      # sign-folded
    cosS = np.concatenate([cos64, cos64], axis=0).astype(np.float32)
    sinS = np.concatenate([sin64, sin64], axis=0).astype(np.float32)

    p = np.arange(128)
    f = np.arange(128)
    tri = (p[:, None] <= f[None, :]).astype(np.float32)  # [128,128] keep
    triS = np.concatenate([tri, tri], axis=1)            # [128, 256]

    in_maps = []
    for c in range(N_CORES):
        b, g = c // 2, c % 2
        rows = slice(DC * g, DC * (g + 1))
        bqk = np.zeros((128, 8), dtype=np.float32)
        for m in range(NM):
            bqk[:, m] = bq_s[rows][128 * m:128 * (m + 1)]
            bqk[:, 4 + m] = bk[rows][128 * m:128 * (m + 1)]
        bf = ml_dtypes.bfloat16
        in_maps.append({
            "xT": np.ascontiguousarray(x[b].T).astype(bf),
            "wqT": np.ascontiguousarray(Wq_s[rows].T).astype(bf),
            "wkT": np.ascontiguousarray(Wk[rows].T).astype(bf),
            "wvT": np.ascontiguousarray(Wv[rows].T).astype(bf),
            "woT": np.ascontiguousarray(Wo[:, rows].T).astype(bf),
            "bqk_cols": bqk,
            "bv_row": bv[rows].reshape(1, DC),
            "ones_col": np.ones((128, 8), dtype=np.float32),
            "cosS": cosS.astype(bf),
            "sinS": sinS.astype(bf),
            "triS": triS.astype(bf),
        })
    return in_maps


def kernel(x, Wq, bq, Wk, bk, Wv, bv, Wo, bo, _trace=False, _tmpdir=None):
    nc = _get_program()
    in_maps = _host_inputs(x, Wq, bq, Wk, bk, Wv, bv, Wo, bo)
    res = run_bass_kernel_spmd(nc, in_maps, list(range(N_CORES)),
                               trace=_trace, tmpdir=_tmpdir)
    kernel.last_exec_time_ns = res.exec_time_ns
    bo = np.asarray(bo, dtype=np.float32)
    out = np.zeros((B, T, D), dtype=np.float32)
    for b in range(B):
        acc = res.results[2 * b]["outT"].astype(np.float32) + \
            res.results[2 * b + 1]["outT"].astype(np.float32)
        out[b] = acc.T + bo[None, :]
    return out
